# revision 8
# baseline (speedup 1.0000x reference)
"""Trainium2 Bass kernel for GQA attention layer (B=1, T=2048, HID=4096,
32 q-heads / 8 kv-heads, head_dim 128, RoPE, causal) sharded over 8 cores.

Sharding: tensor-parallel over heads. Core c owns q-heads 4c..4c+3 and
kv-head c. Attention outputs (transposed, [512 hd, t]) are AllGathered in
four t-chunks (pipelined against later attention compute); each core then
computes a 512-row slice of the output projection over the full 4096 hd
dims, so no AllReduce is needed. Host assembles the 8 output slices.

Matmuls run in bf16 (PE moving operand streams 2B/cycle, so bf16 is 2x
fp32r), accumulation in fp32 PSUM; softmax statistics in fp32.
"""

import numpy as np

import concourse.bacc as bacc
import concourse.mybir as mybir
import concourse.tile as tile
from concourse.bass_utils import run_bass_kernel_spmd

T = 2048
HID = 4096
D = 128
N_HEADS = 32
N_KV = 8
HQ = N_HEADS // N_KV  # q heads per core (=4)
TT = 512  # t tile
NTT = T // TT  # 4
NH = HID // 128  # 32 h-tiles
SCALE = 1.0 / np.sqrt(np.float32(D))
ROPE_BASE = 10000.0
N_CORES = 8

_F32 = mybir.dt.float32
_DT = mybir.dt.bfloat16

_cached = None


def _build():
    nc = bacc.Bacc("TRN2", target_bir_lowering=False, debug=False, num_devices=N_CORES)

    xT = nc.dram_tensor("xT", [HID, T], _DT, kind="ExternalInput").ap()
    wqT = nc.dram_tensor("wqT", [HID, HQ * D], _DT, kind="ExternalInput").ap()
    wkT = nc.dram_tensor("wkT", [HID, D], _DT, kind="ExternalInput").ap()
    wvT = nc.dram_tensor("wvT", [HID, D], _DT, kind="ExternalInput").ap()
    woT = nc.dram_tensor("woT", [HID, HQ * D], _DT, kind="ExternalInput").ap()
    cos2 = nc.dram_tensor("cos2", [128, T], _F32, kind="ExternalInput").ap()
    sinS = nc.dram_tensor("sinS", [128, T], _F32, kind="ExternalInput").ap()
    masks = nc.dram_tensor("masks", [128, 4 * TT], _F32, kind="ExternalInput").ap()
    ones_i = nc.dram_tensor("ones_i", [128, 128], _DT, kind="ExternalInput").ap()
    ident_i = nc.dram_tensor("ident_i", [128, 128], _DT, kind="ExternalInput").ap()
    out = nc.dram_tensor("out", [HQ * D, T], _F32, kind="ExternalOutput").ap()

    Exp = mybir.ActivationFunctionType.Exp

    with tile.TileContext(nc) as tc:
        with (
            tc.tile_pool(name="const", bufs=1) as const,
            tc.tile_pool(name="big", bufs=1) as big,
            tc.tile_pool(name="sb", bufs=1) as sb,
            tc.tile_pool(name="ps", bufs=1, space="PSUM") as ps,
            tc.tile_pool(name="dram", bufs=1, space="DRAM") as dram,
        ):
            # ---- constants / persistent weights in SBUF ----
            cos_sb = const.tile([128, T], _F32, name="cos_sb")
            sin_sb = const.tile([128, T], _F32, name="sin_sb")
            mask_sb = const.tile([128, 4 * TT], _F32, name="mask_sb")
            ones_sb = const.tile([128, 128], _DT, name="ones_sb")
            ident_sb = const.tile([128, 128], _DT, name="ident_sb")
            # per-h-tile weight tiles so the first matmuls only wait on
            # their own slice's DMA, not the whole preload
            wq_t = [const.tile([128, HQ * D], _DT, name=f"wq_t{j}") for j in range(NH)]
            wk_t = [const.tile([128, D], _DT, name=f"wk_t{j}") for j in range(NH)]
            wv_t = [const.tile([128, D], _DT, name=f"wv_t{j}") for j in range(NH)]
            wo_sb = const.tile([128, NH * HQ * D], _DT, name="wo_sb")
            for j in range(NH):
                hsl = slice(128 * j, 128 * (j + 1))
                nc.gpsimd.dma_start(
                    out=wo_sb[:, 512 * j : 512 * (j + 1)], in_=woT[hsl, :]
                )

            nc.scalar.dma_start(out=cos_sb[:], in_=cos2[:])
            nc.scalar.dma_start(out=sin_sb[:], in_=sinS[:])
            nc.scalar.dma_start(out=mask_sb[:], in_=masks[:])
            nc.scalar.dma_start(out=ones_sb[:], in_=ones_i[:])
            nc.scalar.dma_start(out=ident_sb[:], in_=ident_i[:])

            qrot = [big.tile([128, T], _DT, name=f"qrot{h}") for h in range(HQ)]
            krot = big.tile([128, T], _DT, name="krot")
            v_sb = big.tile([128, T], _DT, name="v_sb")  # V[s,d]: block k at cols 128k

            attn_local = [
                dram.tile([HQ * D, TT], _DT, name=f"attn_local{i}") for i in range(NTT)
            ]
            attn_full = [
                dram.tile(
                    [N_CORES * HQ * D, TT],
                    _DT,
                    addr_space="Shared",
                    name=f"attn_full{i}",
                )
                for i in range(NTT)
            ]

            def proj(ti):
                tsl = slice(TT * ti, TT * (ti + 1))
                q_ps = [
                    ps.tile([128, TT], _F32, tag=f"p{h}", name=f"q_ps{h}")
                    for h in range(HQ)
                ]
                k_ps = ps.tile([128, TT], _F32, tag="p4")
                vT_ps = ps.tile([128, TT], _F32, tag="p5")
                for hi in range(NH):
                    hsl = slice(128 * hi, 128 * (hi + 1))
                    if ti == 0:
                        # weight slice DMAs interleaved with x so the first
                        # matmuls aren't queued behind the whole preload
                        nc.sync.dma_start(out=wq_t[hi][:], in_=wqT[hsl, :])
                        nc.sync.dma_start(out=wk_t[hi][:], in_=wkT[hsl, :])
                        nc.sync.dma_start(out=wv_t[hi][:], in_=wvT[hsl, :])
                    xt = sb.tile([128, TT], _DT, tag="x", bufs=12)
                    nc.sync.dma_start(out=xt[:], in_=xT[hsl, tsl])
                    st, sp = hi == 0, hi == NH - 1
                    for h in range(HQ):
                        nc.tensor.matmul(
                            q_ps[h][:],
                            wq_t[hi][:, 128 * h : 128 * (h + 1)],
                            xt[:],
                            start=st,
                            stop=sp,
                        )
                    nc.tensor.matmul(k_ps[:], wk_t[hi][:], xt[:], start=st, stop=sp)
                    nc.tensor.matmul(vT_ps[:], wv_t[hi][:], xt[:], start=st, stop=sp)

                # V: transpose [d, s] -> [s, d] blocks
                vT_sb = sb.tile([128, TT], _DT, tag="vTs", bufs=2)
                nc.vector.tensor_copy(vT_sb[:], vT_ps[:])
                for j in range(TT // 128):
                    vtr = ps.tile([128, 128], _DT, tag="p6", bufs=2)
                    nc.tensor.transpose(
                        vtr[:], vT_sb[:, 128 * j : 128 * (j + 1)], ident_sb[:]
                    )
                    k = (TT // 128) * ti + j
                    nc.vector.tensor_copy(v_sb[:, 128 * k : 128 * (k + 1)], vtr[:])

                # RoPE on q heads and k (fp32 math, bf16 store)
                for h in [0, HQ, 1, 2, 3]:
                    src = q_ps[h] if h < HQ else k_ps
                    qf = sb.tile([128, TT], _F32, tag="qf", bufs=2)
                    nc.vector.tensor_copy(qf[:], src[:])
                    qs = sb.tile([128, TT], _F32, tag="qs", bufs=2)
                    nc.scalar.dma_start(out=qs[0:64, :], in_=qf[64:128, :])
                    nc.scalar.dma_start(out=qs[64:128, :], in_=qf[0:64, :])
                    t1 = sb.tile([128, TT], _F32, tag="t1", bufs=2)
                    nc.vector.tensor_mul(t1[:], qf[:], cos_sb[:, tsl])
                    t2 = sb.tile([128, TT], _F32, tag="t2", bufs=2)
                    nc.vector.tensor_mul(t2[:], qs[:], sin_sb[:, tsl])
                    dst = qrot[h][:, tsl] if h < HQ else krot[:, tsl]
                    nc.vector.tensor_add(dst, t1[:], t2[:])

            def attn(ti):
                nblk = (TT // 128) * (ti + 1)
                for h in range(HQ):
                    attn_ps = ps.tile([128, TT], _F32, tag=f"p{h}")
                    den_ps = ps.tile([128, TT], _F32, tag="p4" if h % 2 == 0 else "p5")
                    sc_tags = [f"p{(h + 1 + j) % HQ}" for j in range(3)] + ["p6"]
                    sc_t = {}
                    probs_t = {}

                    def lo_of(k):
                        diag = k - (TT // 128) * ti
                        return 128 * diag if diag > 0 else 0

                    def emit_sc(k):
                        # scoresT block + exp (ACT); causal sub-range only
                        lo = lo_of(k)
                        diag = k - (TT // 128) * ti
                        qsl = slice(TT * ti + lo, TT * (ti + 1))
                        tg = sc_tags[k % 4]
                        sc = ps.tile(
                            [128, TT],
                            _F32,
                            tag=tg,
                            name=f"sc{k}",
                            bufs=2 if tg == "p6" else 1,
                        )
                        nc.tensor.matmul(
                            sc[:, lo:TT],
                            krot[:, 128 * k : 128 * (k + 1)],
                            qrot[h][:, qsl],
                            start=True,
                            stop=True,
                        )
                        probs = sb.tile([128, TT], _DT, tag="probs", bufs=6)
                        if diag >= 0:
                            ptmp = sb.tile([128, TT], _F32, tag="ptmp", bufs=4)
                            nc.scalar.activation(
                                ptmp[:, lo:TT], sc[:, lo:TT], Exp, scale=SCALE
                            )
                            nc.vector.tensor_mul(
                                probs[:, lo:TT],
                                ptmp[:, lo:TT],
                                mask_sb[:, TT * diag + lo : TT * (diag + 1)],
                            )
                        else:
                            nc.scalar.activation(probs[:], sc[:], Exp, scale=SCALE)
                        probs_t[k] = probs

                    for j in range(min(4, nblk)):
                        emit_sc(j)
                    for k in range(nblk):
                        if k + 4 < nblk:
                            emit_sc(k + 4)
                        lo = lo_of(k)
                        st, sp = k == 0, k == nblk - 1
                        probs = probs_t.pop(k)
                        nc.tensor.matmul(
                            attn_ps[:, lo:TT],
                            v_sb[:, 128 * k : 128 * (k + 1)],
                            probs[:, lo:TT],
                            start=st,
                            stop=sp,
                        )
                        nc.tensor.matmul(
                            den_ps[:, lo:TT],
                            ones_sb[:],
                            probs[:, lo:TT],
                            start=st,
                            stop=sp,
                        )
                    recip = sb.tile([128, TT], _F32, tag="recip", bufs=2)
                    nc.vector.reciprocal_approx_fast(recip[:], den_ps[:])
                    anorm = sb.tile([128, TT], _DT, tag="anorm", bufs=2)
                    nc.vector.tensor_mul(anorm[:], attn_ps[:], recip[:])
                    nc.gpsimd.dma_start(
                        out=attn_local[ti][128 * h : 128 * (h + 1), :], in_=anorm[:]
                    )

            def gather(ti):
                nc.gpsimd.collective_compute(
                    "AllGather",
                    mybir.AluOpType.bypass,
                    replica_groups=[list(range(N_CORES))],
                    ins=[attn_local[ti].opt()],
                    outs=[attn_full[ti].opt()],
                )

            def outproj(ti, tags):
                o_ps = [
                    ps.tile(
                        [128, TT],
                        _F32,
                        tag=tg,
                        name=f"o_ps{ti}_{i}",
                        bufs=2 if tg == "p6" else 1,
                    )
                    for i, tg in enumerate(tags)
                ]
                for hd in range(NH):
                    ag = sb.tile([128, TT], _DT, tag="ag", bufs=8)
                    nc.sync.dma_start(
                        out=ag[:], in_=attn_full[ti][128 * hd : 128 * (hd + 1), :]
                    )
                    st, sp = hd == 0, hd == NH - 1
                    for o in range(4):
                        nc.tensor.matmul(
                            o_ps[o][:],
                            wo_sb[:, 512 * hd + 128 * o : 512 * hd + 128 * (o + 1)],
                            ag[:],
                            start=st,
                            stop=sp,
                        )
                for o in range(4):
                    oc = sb.tile([128, TT], _F32, tag="oc", bufs=4)
                    nc.vector.tensor_copy(oc[:], o_ps[o][:])
                    nc.sync.dma_start(
                        out=out[128 * o : 128 * (o + 1), TT * ti : TT * (ti + 1)],
                        in_=oc[:],
                    )

            # pipeline: AG(ti) overlaps attn(ti+1); outproj(ti) follows
            proj(0)
            attn(0)
            gather(0)
            proj(1)
            attn(1)
            gather(1)
            outproj(0, ["p0", "p1", "p2", "p3"])
            proj(2)
            attn(2)
            gather(2)
            outproj(1, ["p0", "p1", "p2", "p3"])
            proj(3)
            attn(3)
            gather(3)
            outproj(2, ["p0", "p1", "p2", "p3"])
            outproj(3, ["p4", "p5", "p6", "p6"])

    nc.compile()
    return nc


def _host_inputs(hidden_states, Wq, Wk, Wv, Wo):
    import ml_dtypes

    bf16 = ml_dtypes.bfloat16
    x = np.asarray(hidden_states, dtype=np.float32).reshape(T, HID)
    xT = np.ascontiguousarray(x.T).astype(bf16)

    pos = np.arange(T, dtype=np.float32)
    inv_freq = ROPE_BASE ** (-np.arange(0, D, 2, dtype=np.float32) / D)  # [64]
    ang = pos[:, None] * inv_freq[None, :]  # [T, 64]
    cosT = np.cos(ang).T.astype(np.float32)  # [64, T]
    sinT = np.sin(ang).T.astype(np.float32)
    cos2 = np.ascontiguousarray(np.concatenate([cosT, cosT], axis=0))
    sinS = np.ascontiguousarray(np.concatenate([-sinT, sinT], axis=0))

    p = np.arange(128)[:, None]
    tp = np.arange(TT)[None, :]
    masks = np.concatenate(
        [(p <= tp - 128 * j).astype(np.float32) for j in range(4)], axis=1
    )
    masks = np.ascontiguousarray(masks)
    ones = np.ones((128, 128), dtype=bf16)
    ident = np.eye(128, dtype=np.float32).astype(bf16)

    Wq = np.asarray(Wq, dtype=np.float32)
    Wk = np.asarray(Wk, dtype=np.float32)
    Wv = np.asarray(Wv, dtype=np.float32)
    Wo = np.asarray(Wo, dtype=np.float32)

    in_maps = []
    for c in range(N_CORES):
        qs = slice(HQ * D * c, HQ * D * (c + 1))
        ks = slice(D * c, D * (c + 1))
        in_maps.append(
            {
                "xT": xT,
                "wqT": np.ascontiguousarray(Wq[qs, :].T).astype(bf16),
                "wkT": np.ascontiguousarray(Wk[ks, :].T).astype(bf16),
                "wvT": np.ascontiguousarray(Wv[ks, :].T).astype(bf16),
                "woT": np.ascontiguousarray(Wo[qs, :].T).astype(bf16),
                "cos2": cos2,
                "sinS": sinS,
                "masks": masks,
                "ones_i": ones,
                "ident_i": ident,
            }
        )
    return in_maps


def get_program():
    global _cached
    if _cached is None:
        _cached = _build()
    return _cached


def kernel(hidden_states, Wq, Wk, Wv, Wo):
    nc = get_program()
    in_maps = _host_inputs(hidden_states, Wq, Wk, Wv, Wo)
    res = run_bass_kernel_spmd(nc, in_maps, list(range(N_CORES)))
    outT = np.concatenate([res.results[c]["out"] for c in range(N_CORES)], axis=0)
    return np.ascontiguousarray(outT.T).reshape(1, T, HID).astype(np.float32)


# revision 12
# speedup vs baseline: 1.0759x; 1.0759x over previous
"""Trainium2 Bass kernel for GQA attention layer (B=1, T=2048, HID=4096,
32 q-heads / 8 kv-heads, head_dim 128, RoPE, causal) sharded over 8 cores.

Sharding: tensor-parallel over heads. Core c owns q-heads 4c..4c+3 and
kv-head c. Attention outputs (transposed, [512 hd, t]) are AllGathered in
four t-chunks (pipelined against later attention compute); each core then
computes a 512-row slice of the output projection over the full 4096 hd
dims, so no AllReduce is needed. Host assembles the 8 output slices.

Matmuls run in bf16 (PE moving operand streams 2B/cycle, so bf16 is 2x
fp32r), accumulation in fp32 PSUM; softmax statistics in fp32.
"""

import numpy as np

import concourse.bacc as bacc
import concourse.mybir as mybir
import concourse.tile as tile
from concourse.bass_utils import run_bass_kernel_spmd

T = 2048
HID = 4096
D = 128
N_HEADS = 32
N_KV = 8
HQ = N_HEADS // N_KV  # q heads per core (=4)
TT = 512  # t tile
NTT = T // TT  # 4
NH = HID // 128  # 32 h-tiles
SCALE = 1.0 / np.sqrt(np.float32(D))
ROPE_BASE = 10000.0
N_CORES = 8

_F32 = mybir.dt.float32
_DT = mybir.dt.bfloat16

_cached = None


def _build():
    nc = bacc.Bacc("TRN2", target_bir_lowering=False, debug=False, num_devices=N_CORES)

    xT = nc.dram_tensor("xT", [HID, T], _DT, kind="ExternalInput").ap()
    wqkvT = nc.dram_tensor(
        "wqkvT", [HID, (HQ + 2) * D], _DT, kind="ExternalInput"
    ).ap()
    woT = nc.dram_tensor("woT", [HID, HQ * D], _DT, kind="ExternalInput").ap()
    cos2 = nc.dram_tensor("cos2", [128, T], _F32, kind="ExternalInput").ap()
    sinS = nc.dram_tensor("sinS", [128, T], _F32, kind="ExternalInput").ap()
    masks = nc.dram_tensor("masks", [128, 4 * TT], _F32, kind="ExternalInput").ap()
    ones_i = nc.dram_tensor("ones_i", [128, 128], _DT, kind="ExternalInput").ap()
    ident_i = nc.dram_tensor("ident_i", [128, 128], _DT, kind="ExternalInput").ap()
    out = nc.dram_tensor("out", [HQ * D, T], _F32, kind="ExternalOutput").ap()

    Exp = mybir.ActivationFunctionType.Exp

    with tile.TileContext(nc) as tc:
        with (
            tc.tile_pool(name="const", bufs=1) as const,
            tc.tile_pool(name="big", bufs=1) as big,
            tc.tile_pool(name="sb", bufs=1) as sb,
            tc.tile_pool(name="ps", bufs=1, space="PSUM") as ps,
            tc.tile_pool(name="dram", bufs=1, space="DRAM") as dram,
        ):
            # ---- constants / persistent weights in SBUF ----
            cos_sb = const.tile([128, T], _F32, name="cos_sb")
            sin_sb = const.tile([128, T], _F32, name="sin_sb")
            mask_sb = const.tile([128, 4 * TT], _F32, name="mask_sb")
            ones_sb = const.tile([128, 128], _DT, name="ones_sb")
            ident_sb = const.tile([128, 128], _DT, name="ident_sb")
            # per-h-tile weight tiles so the first matmuls only wait on
            # their own slice's DMA, not the whole preload
            wqkv_t = [
                const.tile([128, (HQ + 2) * D], _DT, name=f"wqkv_t{j}")
                for j in range(NH)
            ]
            wo_sb = const.tile([128, NH * HQ * D], _DT, name="wo_sb")

            nc.scalar.dma_start(out=cos_sb[:], in_=cos2[:])
            nc.scalar.dma_start(out=sin_sb[:], in_=sinS[:])
            nc.scalar.dma_start(out=mask_sb[:], in_=masks[:])
            nc.scalar.dma_start(out=ones_sb[:], in_=ones_i[:])
            nc.scalar.dma_start(out=ident_sb[:], in_=ident_i[:])

            qrot = [big.tile([128, T], _DT, name=f"qrot{h}") for h in range(HQ)]
            krot = big.tile([128, T], _DT, name="krot")
            v_sb = big.tile([128, T], _DT, name="v_sb")  # V[s,d]: block k at cols 128k

            attn_local = [
                dram.tile([HQ * D, TT], _DT, name=f"attn_local{i}") for i in range(NTT)
            ]
            attn_full = [
                dram.tile(
                    [N_CORES * HQ * D, TT],
                    _DT,
                    addr_space="Shared",
                    name=f"attn_full{i}",
                )
                for i in range(NTT)
            ]

            def proj(ti):
                tsl = slice(TT * ti, TT * (ti + 1))
                q_ps = [
                    ps.tile([128, TT], _F32, tag=f"p{h}", name=f"q_ps{h}")
                    for h in range(HQ)
                ]
                k_ps = ps.tile([128, TT], _F32, tag="p4")
                vT_ps = ps.tile([128, TT], _F32, tag="p5")
                for hi in range(NH):
                    hsl = slice(128 * hi, 128 * (hi + 1))
                    if ti == 0:
                        # weight slice DMA interleaved with x so the first
                        # matmuls aren't queued behind the whole preload
                        nc.sync.dma_start(out=wqkv_t[hi][:], in_=wqkvT[hsl, :])
                    xt = sb.tile([128, TT], _DT, tag="x", bufs=12)
                    nc.sync.dma_start(out=xt[:], in_=xT[hsl, tsl])
                    st, sp = hi == 0, hi == NH - 1
                    for h in range(HQ):
                        nc.tensor.matmul(
                            q_ps[h][:],
                            wqkv_t[hi][:, 128 * h : 128 * (h + 1)],
                            xt[:],
                            start=st,
                            stop=sp,
                        )
                    nc.tensor.matmul(
                        k_ps[:],
                        wqkv_t[hi][:, HQ * D : (HQ + 1) * D],
                        xt[:],
                        start=st,
                        stop=sp,
                    )
                    nc.tensor.matmul(
                        vT_ps[:],
                        wqkv_t[hi][:, (HQ + 1) * D : (HQ + 2) * D],
                        xt[:],
                        start=st,
                        stop=sp,
                    )

                # V: transpose [d, s] -> [s, d] blocks
                vT_sb = sb.tile([128, TT], _DT, tag="vTs", bufs=2)
                nc.vector.tensor_copy(vT_sb[:], vT_ps[:])
                for j in range(TT // 128):
                    vtr = ps.tile([128, 128], _DT, tag="p6", bufs=2)
                    nc.tensor.transpose(
                        vtr[:], vT_sb[:, 128 * j : 128 * (j + 1)], ident_sb[:]
                    )
                    k = (TT // 128) * ti + j
                    nc.vector.tensor_copy(v_sb[:, 128 * k : 128 * (k + 1)], vtr[:])

                # RoPE on q heads and k (fp32 math, bf16 store)
                for h in [0, HQ, 1, 2, 3]:
                    src = q_ps[h] if h < HQ else k_ps
                    qf = sb.tile([128, TT], _F32, tag="qf", bufs=2)
                    nc.vector.tensor_copy(qf[:], src[:])
                    qs = sb.tile([128, TT], _F32, tag="qs", bufs=2)
                    nc.scalar.dma_start(out=qs[0:64, :], in_=qf[64:128, :])
                    nc.scalar.dma_start(out=qs[64:128, :], in_=qf[0:64, :])
                    t1 = sb.tile([128, TT], _F32, tag="t1", bufs=2)
                    nc.vector.tensor_mul(t1[:], qf[:], cos_sb[:, tsl])
                    t2 = sb.tile([128, TT], _F32, tag="t2", bufs=2)
                    nc.vector.tensor_mul(t2[:], qs[:], sin_sb[:, tsl])
                    dst = qrot[h][:, tsl] if h < HQ else krot[:, tsl]
                    nc.vector.tensor_add(dst, t1[:], t2[:])

            def attn(ti, split_gather=False):
                nblk = (TT // 128) * (ti + 1)
                for h in range(HQ):
                    attn_ps = ps.tile([128, TT], _F32, tag=f"p{h}")
                    den_ps = ps.tile([128, TT], _F32, tag="p4" if h % 2 == 0 else "p5")
                    sc_tags = [f"p{(h + 1 + j) % HQ}" for j in range(3)] + ["p6"]
                    sc_t = {}
                    probs_t = {}

                    def lo_of(k):
                        diag = k - (TT // 128) * ti
                        return 128 * diag if diag > 0 else 0

                    def emit_sc(k):
                        # scoresT block + exp (ACT); causal sub-range only
                        lo = lo_of(k)
                        diag = k - (TT // 128) * ti
                        qsl = slice(TT * ti + lo, TT * (ti + 1))
                        tg = sc_tags[k % 4]
                        sc = ps.tile(
                            [128, TT],
                            _F32,
                            tag=tg,
                            name=f"sc{k}",
                            bufs=2 if tg == "p6" else 1,
                        )
                        nc.tensor.matmul(
                            sc[:, lo:TT],
                            krot[:, 128 * k : 128 * (k + 1)],
                            qrot[h][:, qsl],
                            start=True,
                            stop=True,
                        )
                        probs = sb.tile([128, TT], _DT, tag="probs", bufs=6)
                        if diag >= 0:
                            ptmp = sb.tile([128, TT], _F32, tag="ptmp", bufs=4)
                            nc.scalar.activation(
                                ptmp[:, lo:TT], sc[:, lo:TT], Exp, scale=SCALE
                            )
                            nc.vector.tensor_mul(
                                probs[:, lo:TT],
                                ptmp[:, lo:TT],
                                mask_sb[:, TT * diag + lo : TT * (diag + 1)],
                            )
                        else:
                            nc.scalar.activation(probs[:], sc[:], Exp, scale=SCALE)
                        probs_t[k] = probs

                    for j in range(min(4, nblk)):
                        emit_sc(j)
                    for k in range(nblk):
                        if k + 4 < nblk:
                            emit_sc(k + 4)
                        lo = lo_of(k)
                        st, sp = k == 0, k == nblk - 1
                        probs = probs_t.pop(k)
                        nc.tensor.matmul(
                            attn_ps[:, lo:TT],
                            v_sb[:, 128 * k : 128 * (k + 1)],
                            probs[:, lo:TT],
                            start=st,
                            stop=sp,
                        )
                        nc.tensor.matmul(
                            den_ps[:, lo:TT],
                            ones_sb[:],
                            probs[:, lo:TT],
                            start=st,
                            stop=sp,
                        )
                    recip = sb.tile([128, TT], _F32, tag="recip", bufs=2)
                    nc.vector.reciprocal_approx_fast(recip[:], den_ps[:])
                    anorm = sb.tile([128, TT], _DT, tag="anorm", bufs=2)
                    nc.vector.tensor_mul(anorm[:], attn_ps[:], recip[:])
                    nc.sync.dma_start(
                        out=attn_local[ti][128 * h : 128 * (h + 1), :], in_=anorm[:]
                    )
                    if split_gather and h == 1:
                        gather_half(ti, 0)

            def gather(ti):
                nc.gpsimd.collective_compute(
                    "AllGather",
                    mybir.AluOpType.bypass,
                    replica_groups=[list(range(N_CORES))],
                    ins=[attn_local[ti].opt()],
                    outs=[attn_full[ti].opt()],
                )

            # the last t-chunk is gathered in two half-gathers (heads 0-1,
            # then 2-3) so the final output projection can start earlier
            attn_half = [
                dram.tile(
                    [N_CORES * 2 * D, TT], _DT, addr_space="Shared", name=f"attn_h{i}"
                )
                for i in range(2)
            ]

            def gather_half(ti, half):
                nc.gpsimd.collective_compute(
                    "AllGather",
                    mybir.AluOpType.bypass,
                    replica_groups=[list(range(N_CORES))],
                    ins=[attn_local[ti][256 * half : 256 * (half + 1), :]],
                    outs=[attn_half[half].opt()],
                )

            def load_wo():
                for j in range(NH):
                    hsl = slice(128 * j, 128 * (j + 1))
                    nc.gpsimd.dma_start(
                        out=wo_sb[:, 512 * j : 512 * (j + 1)], in_=woT[hsl, :]
                    )

            def outproj(ti, tags):
                o_ps = [
                    ps.tile(
                        [128, TT],
                        _F32,
                        tag=tg,
                        name=f"o_ps{ti}_{i}",
                        bufs=2 if tg == "p6" else 1,
                    )
                    for i, tg in enumerate(tags)
                ]
                for hd in range(NH):
                    ag = sb.tile([128, TT], _DT, tag="ag", bufs=8)
                    nc.sync.dma_start(
                        out=ag[:], in_=attn_full[ti][128 * hd : 128 * (hd + 1), :]
                    )
                    st, sp = hd == 0, hd == NH - 1
                    for o in range(4):
                        nc.tensor.matmul(
                            o_ps[o][:],
                            wo_sb[:, 512 * hd + 128 * o : 512 * hd + 128 * (o + 1)],
                            ag[:],
                            start=st,
                            stop=sp,
                        )
                for o in range(4):
                    oc = sb.tile([128, TT], _F32, tag="oc", bufs=4)
                    nc.vector.tensor_copy(oc[:], o_ps[o][:])
                    nc.sync.dma_start(
                        out=out[128 * o : 128 * (o + 1), TT * ti : TT * (ti + 1)],
                        in_=oc[:],
                    )

            def outproj3():
                tags = ["p4", "p5", "p6", "p6"]
                o_ps = [
                    ps.tile(
                        [128, TT],
                        _F32,
                        tag=tg,
                        name=f"o_ps3_{i}",
                        bufs=2 if tg == "p6" else 1,
                    )
                    for i, tg in enumerate(tags)
                ]
                first = True
                for half in range(2):
                    for r in range(N_CORES):
                        for hp in range(2):
                            g = 4 * r + 2 * half + hp
                            row = 256 * r + 128 * hp
                            ag = sb.tile([128, TT], _DT, tag="ag", bufs=8)
                            nc.sync.dma_start(
                                out=ag[:], in_=attn_half[half][row : row + 128, :]
                            )
                            sp = half == 1 and r == N_CORES - 1 and hp == 1
                            for o in range(4):
                                nc.tensor.matmul(
                                    o_ps[o][:],
                                    wo_sb[
                                        :, 512 * g + 128 * o : 512 * g + 128 * (o + 1)
                                    ],
                                    ag[:],
                                    start=first,
                                    stop=sp,
                                )
                            first = False
                for o in range(4):
                    oc = sb.tile([128, TT], _F32, tag="oc", bufs=4)
                    nc.vector.tensor_copy(oc[:], o_ps[o][:])
                    nc.sync.dma_start(
                        out=out[128 * o : 128 * (o + 1), 3 * TT : 4 * TT], in_=oc[:]
                    )

            # pipeline: AG(ti) overlaps attn(ti+1); outproj(ti) follows
            proj(0)
            attn(0)
            gather(0)
            load_wo()
            proj(1)
            attn(1)
            gather(1)
            outproj(0, ["p0", "p1", "p2", "p3"])
            proj(2)
            attn(2)
            gather(2)
            outproj(1, ["p0", "p1", "p2", "p3"])
            proj(3)
            attn(3, split_gather=True)
            gather_half(3, 1)
            outproj(2, ["p0", "p1", "p2", "p3"])
            outproj3()

    nc.compile()
    return nc


def _host_inputs(hidden_states, Wq, Wk, Wv, Wo):
    import ml_dtypes

    bf16 = ml_dtypes.bfloat16
    x = np.asarray(hidden_states, dtype=np.float32).reshape(T, HID)
    xT = np.ascontiguousarray(x.T).astype(bf16)

    pos = np.arange(T, dtype=np.float32)
    inv_freq = ROPE_BASE ** (-np.arange(0, D, 2, dtype=np.float32) / D)  # [64]
    ang = pos[:, None] * inv_freq[None, :]  # [T, 64]
    cosT = np.cos(ang).T.astype(np.float32)  # [64, T]
    sinT = np.sin(ang).T.astype(np.float32)
    cos2 = np.ascontiguousarray(np.concatenate([cosT, cosT], axis=0))
    sinS = np.ascontiguousarray(np.concatenate([-sinT, sinT], axis=0))

    p = np.arange(128)[:, None]
    tp = np.arange(TT)[None, :]
    masks = np.concatenate(
        [(p <= tp - 128 * j).astype(np.float32) for j in range(4)], axis=1
    )
    masks = np.ascontiguousarray(masks)
    ones = np.ones((128, 128), dtype=bf16)
    ident = np.eye(128, dtype=np.float32).astype(bf16)

    Wq = np.asarray(Wq, dtype=np.float32)
    Wk = np.asarray(Wk, dtype=np.float32)
    Wv = np.asarray(Wv, dtype=np.float32)
    Wo = np.asarray(Wo, dtype=np.float32)

    in_maps = []
    for c in range(N_CORES):
        qs = slice(HQ * D * c, HQ * D * (c + 1))
        ks = slice(D * c, D * (c + 1))
        in_maps.append(
            {
                "xT": xT,
                "wqkvT": np.ascontiguousarray(
                    np.concatenate(
                        [Wq[qs, :].T, Wk[ks, :].T, Wv[ks, :].T], axis=1
                    )
                ).astype(bf16),
                "woT": np.ascontiguousarray(Wo[qs, :].T).astype(bf16),
                "cos2": cos2,
                "sinS": sinS,
                "masks": masks,
                "ones_i": ones,
                "ident_i": ident,
            }
        )
    return in_maps


def get_program():
    global _cached
    if _cached is None:
        _cached = _build()
    return _cached


def kernel(hidden_states, Wq, Wk, Wv, Wo):
    nc = get_program()
    in_maps = _host_inputs(hidden_states, Wq, Wk, Wv, Wo)
    res = run_bass_kernel_spmd(nc, in_maps, list(range(N_CORES)))
    outT = np.concatenate([res.results[c]["out"] for c in range(N_CORES)], axis=0)
    return np.ascontiguousarray(outT.T).reshape(1, T, HID).astype(np.float32)
